# revision 17
# baseline (speedup 1.0000x reference)
"""Trainium2 Bass kernel for a discriminative (pull/push/reg) segmentation loss.

Contract: kernel(embedding_maps, instance_masks) -> scalar np.float32
  embedding_maps: [4, 16, 512, 512] float32
  instance_masks: [4, 12, 512, 512] int32 (0/1)

Sharding: 8 cores = 4 images x 2 instance-halves (6 instances each).
Each core computes, for its 6 masks over the full image:
  counts_k, sums_kd -> means, and pull_sum_k = sum_p m*(relu(dist-0.5))^2
entirely on device.  The host combines the tiny per-core outputs
(means/counts/pull) into the final pull/push/reg scalar.

Device data layout ("pixel stack"): pixels are grouped 1024 at a time
(8 chunks x 128).  e_stack[(c,d), g*128+q] = E[d, g*1024+c*128+q]  (bf16)
m_t[q, g*48+c*6+k] = mask[k, g*1024+c*128+q]                       (bf16)

Pass 1 streams transposed E tiles (DMA xbar transpose) to accumulate
masked sums / counts on the TensorEngine and per-pixel |E|^2.
A small stats phase forms means / -2*means blockdiag / |mean|^2.
Pass 2 streams e_stack to build d2 = |E|^2 - 2 E.mu + |mu|^2 per
(pixel, instance) wholly in PSUM via 2 matmuls, then
G = relu(sqrt(relu(d2)*m) - 0.5) via one fused DVE op + 2 ACT ops, and
pull sums via G^T G matmuls (diagonal extracted on host).
"""

import numpy as np
import ml_dtypes

# ---- problem constants (hardcoded per contract) ----
B, D, H, W = 4, 16, 512, 512
K = 12
KH = 6                  # instances per core
NCORES = 8
HW = H * W              # 262144 pixels
P = 128                 # SBUF partitions
NCH = 8                 # pixel chunks per group
QP = 128                # pixels per chunk
GPX = NCH * QP          # 1024 pixels per group
NG = HW // GPX          # 256 groups
GM = 8                  # groups per macro tile
NMAC = NG // GM         # 32 macro iterations
CD = NCH * D            # 128   (c,d) stacked rows
CK = NCH * KH           # 48    (c,k) stacked cols
DELTA_PULL = 0.5
DELTA_PUSH = 1.5

_CACHE = {}


def _build_program():
    import concourse.bass as bass
    import concourse.tile as tile
    from concourse import bacc, mybir
    from contextlib import ExitStack

    import concourse.bass as _bass

    def _make_bcast_ap(src_ap):
        # [D, KH] -> [D, NCH(bcast), KH]
        return _bass.AP(
            tensor=src_ap.tensor, offset=src_ap.offset,
            ap=[src_ap.ap[0], [0, NCH], src_ap.ap[1]],
        )

    f32 = mybir.dt.float32
    bf16 = mybir.dt.bfloat16
    AX = mybir.AxisListType
    OP = mybir.AluOpType
    AF = mybir.ActivationFunctionType

    nc = bacc.Bacc()

    e_stack = nc.declare_dram_parameter("e_stack", [P, NG * QP], bf16, isOutput=False)
    m_t_d = nc.declare_dram_parameter("m_t", [P, NG * CK], bf16, isOutput=False)
    ident6_d = nc.declare_dram_parameter("ident6", [KH, KH], f32, isOutput=False)
    tiled16_d = nc.declare_dram_parameter("tiled16", [D, CD], f32, isOutput=False)
    tid_cd_d_d = nc.declare_dram_parameter("tid_cd_d", [CD, D], f32, isOutput=False)
    tid6_d = nc.declare_dram_parameter("tid6", [CK, KH], f32, isOutput=False)
    blockmask_d = nc.declare_dram_parameter("blockmask", [CD, CK], bf16, isOutput=False)
    identp_d = nc.declare_dram_parameter("identp", [P, P], bf16, isOutput=False)
    out_pull = nc.declare_dram_parameter("out_pull", [CK, CK], f32, isOutput=True)
    out_counts = nc.declare_dram_parameter("out_counts", [CK, 1], f32, isOutput=True)
    out_means = nc.declare_dram_parameter("out_means", [KH, D], f32, isOutput=True)

    with ExitStack() as ctx:
        tc = ctx.enter_context(tile.TileContext(nc))
        persist = ctx.enter_context(tc.tile_pool(name="persist", bufs=1))
        rot = ctx.enter_context(tc.tile_pool(name="rot", bufs=3))

        chain = ctx.enter_context(tc.tile_pool(name="chain", bufs=2))
        psum_per = ctx.enter_context(tc.tile_pool(name="psum_per", bufs=1, space="PSUM"))
        psum_rot = ctx.enter_context(tc.tile_pool(name="psum_rot", bufs=2, space="PSUM"))
        psum_tiny = ctx.enter_context(tc.tile_pool(name="psum_tiny", bufs=1, space="PSUM"))

        # persistent tiles
        e_res = persist.tile([P, NG * QP], bf16)        # resident e_stack
        identp = persist.tile([P, P], bf16)
        m_t_res = persist.tile([P, NG * CK], bf16)      # all masks, pixel-major
        embsq = persist.tile([P, NG * NCH], f32)        # per-pixel |E|^2, col g*8+c
        ones_q = persist.tile([P, 1], bf16)
        ones_row = persist.tile([1, P], f32)
        bd = persist.tile([CD, CK], bf16)               # blockdiag(-2*means)
        msq_row = persist.tile([1, CK], f32)            # |mean_k|^2 tiled over c
        neg_delta = persist.tile([P, 1], f32)
        ident6 = persist.tile([KH, KH], f32)
        tiled16 = persist.tile([D, CD], f32)
        tid_cd_d = persist.tile([CD, D], f32)
        tid6 = persist.tile([CK, KH], f32)
        blockmask = persist.tile([CD, CK], bf16)
        warm = persist.tile([1, 1], f32)

        nc.vector.memset(ones_q[:], 1.0)
        nc.vector.memset(ones_row[:], 1.0)
        nc.vector.memset(neg_delta[:], -DELTA_PULL)
        nc.sync.dma_start(ident6[:], ident6_d[:])
        nc.sync.dma_start(tiled16[:], tiled16_d[:])
        nc.sync.dma_start(tid_cd_d[:], tid_cd_d_d[:])
        nc.sync.dma_start(tid6[:], tid6_d[:])
        nc.sync.dma_start(blockmask[:], blockmask_d[:])
        nc.sync.dma_start(identp[:], identp_d[:])
        # engine warm-ups: make ACT/PE observe the const/memset ticks once so
        # later instructions need at most 2 sync waits (ISA limit).
        nc.scalar.activation(warm[:], ones_row[0:1, 0:1], AF.Square)

        psum_sums = psum_per.tile([CD, CK], f32)
        psum_counts = psum_per.tile([CK, 1], f32)
        psum_pull = psum_per.tile([CK, CK], f32)

        # ---------------- bulk loads (all DMA up front, dependency-free) ----
        NEQ = 8
        for i in range(NEQ):
            s = slice(i * NG * QP // NEQ, (i + 1) * NG * QP // NEQ)
            nc.sync.dma_start(e_res[:, s], e_stack[:, s])
        NMQ = 4
        for i in range(NMQ):
            s = slice(i * NG * CK // NMQ, (i + 1) * NG * CK // NMQ)
            nc.sync.dma_start(m_t_res[:, s], m_t_d[:, s])

        # ---------------- pass 1: masked sums / counts / |E|^2 ----------------
        for m in range(NMAC):
            e_t = rot.tile([P, GM, CD], bf16, tag="e_t")    # [q, g, (c,d)]
            for g in range(GM):
                gg = m * GM + g
                tps = psum_rot.tile([P, P], bf16, tag="pP")
                nc.tensor.transpose(
                    tps[:], e_res[:, gg * QP:(gg + 1) * QP], identp[:]
                )
                if g % 2 == 0:
                    nc.vector.tensor_copy(e_t[:, g, :], tps[:])
                else:
                    nc.scalar.activation(e_t[:, g, :], tps[:], AF.Copy)

            e_t_flat = e_t[:].rearrange("p g r -> p (g r)")
            sq_t = rot.tile([P, GM * CD], bf16, tag="sq_t")
            nc.scalar.activation(sq_t[:], e_t_flat, AF.Square)
            nc.vector.tensor_reduce(
                out=embsq[:, m * GM * NCH:(m + 1) * GM * NCH],
                in_=sq_t[:].rearrange("p (a d) -> p a d", d=D),
                axis=AX.X,
                op=OP.add,
            )
            for g in range(GM):
                gg = m * GM + g
                mgs = slice(gg * CK, (gg + 1) * CK)
                nc.tensor.matmul(
                    psum_sums[:], e_t[:, g, :], m_t_res[:, mgs],
                    start=(gg == 0), stop=(gg == NG - 1),
                )
                nc.tensor.matmul(
                    psum_counts[:], m_t_res[:, mgs], ones_q[:],
                    start=(gg == 0), stop=(gg == NG - 1),
                )

        # ---------------- stats: means, blockdiag, |mean|^2 ----------------
        # (no cross-partition DMAs: fold diag blocks with masks + tiny matmuls)
        sums_sb = persist.tile([CD, CK], f32)
        nc.vector.tensor_copy(sums_sb[:], psum_sums[:])
        counts_sb = persist.tile([CK, 1], f32)
        nc.vector.tensor_copy(counts_sb[:], psum_counts[:])

        # keep only diagonal (c) blocks, fold over c in the free dim
        s_diag = persist.tile([CD, CK], f32)
        nc.vector.tensor_mul(s_diag[:], sums_sb[:], blockmask[:])
        s_fold = persist.tile([CD, KH], f32)
        nc.vector.tensor_reduce(
            out=s_fold[:],
            in_=s_diag[:].rearrange("p (c k) -> p k c", c=NCH),
            axis=AX.X, op=OP.add,
        )
        # sums_kd[k, d] = sum_c s_fold[(c,d), k] via tiled-identity contraction
        psum_kd = psum_tiny.tile([KH, D], f32, tag="ptx")
        nc.tensor.matmul(psum_kd[:], s_fold[:], tid_cd_d[:], start=True, stop=True)

        # counts_k[k] = sum_c counts[(c,k)]
        psum_ck = psum_tiny.tile([KH, 1], f32, tag="pty")
        nc.tensor.matmul(psum_ck[:], tid6[:], counts_sb[:], start=True, stop=True)
        counts_k = persist.tile([KH, 1], f32)
        nc.vector.tensor_copy(counts_k[:], psum_ck[:])
        safe_k = persist.tile([KH, 1], f32)
        nc.vector.tensor_scalar_max(safe_k[:], counts_k[:], 1.0)
        recip_k = persist.tile([KH, 1], f32)
        nc.vector.reciprocal(recip_k[:], safe_k[:])

        means_kd = persist.tile([KH, D], f32)
        nc.vector.tensor_scalar(
            out=means_kd[:], in0=psum_kd[:], scalar1=recip_k[:], scalar2=None,
            op0=OP.mult,
        )
        nc.gpsimd.dma_start(out_means[:], means_kd[:])

        # means_dk = means_kd.T (PE transpose, base partition 0)
        psum_dk = psum_tiny.tile([D, KH], f32, tag="ptx")
        nc.tensor.transpose(psum_dk[:], means_kd[:], ident6[:])
        mdk_sb = persist.tile([D, KH], f32)
        nc.vector.tensor_scalar(
            out=mdk_sb[:], in0=psum_dk[:], scalar1=-2.0, scalar2=None, op0=OP.mult
        )
        # bd = blockdiag(-2*means): dense replicate via matmul, then mask
        psum_dense = psum_tiny.tile([CD, CK], f32, tag="pty")
        src_ap = mdk_sb[:]
        mdk_b = _make_bcast_ap(src_ap)
        nc.tensor.matmul(psum_dense[:], tiled16[:], mdk_b, start=True, stop=True)
        nc.vector.tensor_mul(bd[:], psum_dense[:], blockmask[:])

        # msq_row[0, (c,k)] = |mean_k|^2
        msq_t = persist.tile([KH, D], f32)
        nc.vector.tensor_mul(msq_t[:], means_kd[:], means_kd[:])
        msq_k = persist.tile([KH, 1], f32)
        nc.vector.tensor_reduce(out=msq_k[:], in_=msq_t[:], axis=AX.X, op=OP.add)
        psum_mr = psum_tiny.tile([1, KH], f32, tag="ptx")
        nc.tensor.transpose(psum_mr[:], msq_k[:], ident6[:])
        mr_src = psum_mr[:]
        mr_b = _bass.AP(
            tensor=mr_src.tensor, offset=mr_src.offset,
            ap=[mr_src.ap[0], [0, NCH], mr_src.ap[1]],
        )
        nc.vector.tensor_copy(msq_row[:].rearrange("p (c k) -> p c k", c=NCH), mr_b)

        # ---------------- pass 2: d2 -> pull sums ----------------
        for m in range(NMAC):
            msl = slice(m * GM * CK, (m + 1) * GM * CK)

            pP = psum_rot.tile([P, GM * CK], f32, tag="pP")
            for g in range(GM):
                gg = m * GM + g
                sl = slice(g * CK, (g + 1) * CK)
                nc.tensor.matmul(
                    pP[:, sl], ones_row[:], msq_row[:], start=True, stop=False
                )
                nc.tensor.matmul(
                    pP[:, sl], e_res[:, gg * QP:(gg + 1) * QP], bd[:],
                    start=False, stop=True,
                )

            # t = d2 = P + embsq (broadcast over k)
            eb = embsq[:, m * GM * NCH:(m + 1) * GM * NCH]
            eb_b = _bass.AP(
                tensor=eb.tensor, offset=eb.offset,
                ap=[eb.ap[0], eb.ap[1], [0, KH]],
            )  # [p, (g c), k]
            t_t = chain.tile([P, GM * CK], f32, tag="t_t")
            nc.vector.scalar_tensor_tensor(
                out=t_t[:].rearrange("p (a k) -> p a k", k=KH),
                in0=pP[:].rearrange("p (a k) -> p a k", k=KH),
                scalar=0.0, in1=eb_b, op0=OP.bypass, op1=OP.add,
            )
            # u = relu(d2) * m
            u_t = chain.tile([P, GM * CK], f32, tag="u_t")
            nc.vector.scalar_tensor_tensor(
                out=u_t[:], in0=t_t[:], scalar=0.0, in1=m_t_res[:, msl],
                op0=OP.max, op1=OP.mult,
            )
            w_t = chain.tile([P, GM * CK], f32, tag="w_t")
            nc.scalar.sqrt(w_t[:], u_t[:])
            g_t = chain.tile([P, GM * CK], f32, tag="g_t")
            nc.scalar.activation(g_t[:], w_t[:], AF.Relu, bias=neg_delta[:])
            for g in range(GM):
                gg = m * GM + g
                sl = slice(g * CK, (g + 1) * CK)
                nc.tensor.matmul(
                    psum_pull[:], g_t[:, sl], g_t[:, sl],
                    start=(gg == 0), stop=(gg == NG - 1),
                )

        pull_sb = persist.tile([CK, CK], f32)
        nc.vector.tensor_copy(pull_sb[:], psum_pull[:])
        nc.gpsimd.dma_start(out_pull[:], pull_sb[:])
        nc.gpsimd.dma_start(out_counts[:], counts_sb[:])

    nc.finalize()
    return nc


def _get_program():
    if "nc" not in _CACHE:
        _CACHE["nc"] = _build_program()
    return _CACHE["nc"]


def _host_consts():
    bf = ml_dtypes.bfloat16
    id16 = np.eye(D, dtype=np.float32)
    id6 = np.eye(KH, dtype=np.float32)
    tiled16 = np.tile(id16, (1, NCH)).astype(np.float32)          # [16, 128]
    tid_cd_d = np.tile(id16, (NCH, 1)).astype(np.float32)         # [128, 16]
    tid6 = np.tile(id6, (NCH, 1)).astype(np.float32)              # [48, 6]
    blockmask = np.zeros((CD, CK), dtype=np.float32)
    for c in range(NCH):
        blockmask[c * D:(c + 1) * D, c * KH:(c + 1) * KH] = 1.0
    return {
        "ident6": id6,
        "tiled16": tiled16,
        "tid_cd_d": tid_cd_d,
        "tid6": tid6,
        "blockmask": blockmask.astype(bf),
        "identp": np.eye(P, dtype=np.float32).astype(bf),
    }


def _prep_core_inputs(emb_b, masks_bh):
    """emb_b: [16, H, W] f32; masks_bh: [6, H, W] (int or float)."""
    bf = ml_dtypes.bfloat16
    e = np.ascontiguousarray(emb_b.reshape(D, NG, NCH, QP).transpose(2, 0, 1, 3))
    e_stack = e.reshape(P, NG * QP).astype(bf)
    mk = masks_bh.reshape(KH, NG, NCH, QP).transpose(3, 1, 2, 0)  # [q, g, c, k]
    m_t = np.ascontiguousarray(mk).reshape(P, NG * CK).astype(bf)
    return {"e_stack": e_stack, "m_t": m_t, **_host_consts()}


def _host_combine(core_results):
    """core_results: list of 8 dicts with out_pull [48,48], out_counts [48,1],
    out_means [6,16]. Returns np.float32 scalar total loss."""
    total = 0.0
    for b in range(B):
        means = []
        counts = []
        pull_sums = []
        for h in range(2):
            r = core_results[b * 2 + h]
            pull_k = np.diag(r["out_pull"].astype(np.float64)).reshape(NCH, KH).sum(0)
            cnt_k = r["out_counts"].astype(np.float64).reshape(NCH, KH).sum(0)
            means.append(r["out_means"].astype(np.float64))
            counts.append(cnt_k)
            pull_sums.append(pull_k)
        means = np.concatenate(means, 0)          # [12, 16]
        counts = np.concatenate(counts, 0)        # [12]
        pull_sums = np.concatenate(pull_sums, 0)  # [12]

        valid = counts > 0
        validf = valid.astype(np.float64)
        safe_counts = np.maximum(counts, 1.0)
        nv = validf.sum()
        safe_nv = max(nv, 1.0)

        pull_k = pull_sums / safe_counts
        pull_img = (pull_k * validf).sum() / safe_nv if nv > 0 else 0.0

        mean_sq = (means * means).sum(-1)                      # [12]
        cross = means @ means.T                                # [12,12]
        pd2 = np.maximum(mean_sq[:, None] + mean_sq[None, :] - 2.0 * cross, 0.0)
        iu = np.triu_indices(K, k=1)
        pair_mask = (valid[:, None] & valid[None, :])[iu]
        pdist = np.sqrt(pd2[iu])
        push_terms = np.where(
            pair_mask, np.maximum(2.0 * DELTA_PUSH - pdist, 0.0) ** 2, 0.0
        )
        n_pairs = nv * (nv - 1.0) / 2.0
        push_img = push_terms.sum() / max(n_pairs, 1.0) if nv > 1 else 0.0

        reg_img = (np.sqrt(mean_sq) * validf).sum() / safe_nv if nv > 0 else 0.0

        total += pull_img + push_img + reg_img
    return np.float32(total / B)


def kernel(embedding_maps, instance_masks):
    from concourse.bass_utils import run_bass_kernel_spmd

    nc = _get_program()
    emb = np.asarray(embedding_maps, dtype=np.float32)
    msk = np.asarray(instance_masks)

    in_maps = []
    for core in range(NCORES):
        b, h = core // 2, core % 2
        in_maps.append(
            _prep_core_inputs(emb[b], msk[b, h * KH:(h + 1) * KH].astype(np.float32))
        )

    res = run_bass_kernel_spmd(nc, in_maps, list(range(NCORES)))
    return _host_combine(res.results)


if __name__ == "__main__":
    rng = np.random.default_rng(0)
    emb = rng.standard_normal((B, D, H, W), dtype=np.float32)
    msk = (rng.random((B, K, H, W)) < 0.5).astype(np.int32)
    print(kernel(emb, msk))


# revision 18
# speedup vs baseline: 64728.1307x; 64728.1307x over previous
"""Trainium2 Bass kernel for a discriminative (pull/push/reg) segmentation loss.

Contract: kernel(embedding_maps, instance_masks) -> scalar np.float32
  embedding_maps: [4, 16, 512, 512] float32
  instance_masks: [4, 12, 512, 512] int32 (0/1)

Sharding: 8 cores = 4 images x 2 instance-halves (6 instances each).
Each core computes, for its 6 masks over the full image:
  counts_k, sums_kd -> means, and pull_sum_k = sum_p m*(relu(dist-0.5))^2
entirely on device.  The host combines the tiny per-core outputs
(means/counts/pull) into the final pull/push/reg scalar.

Device data layout ("pixel stack"): pixels are grouped 1024 at a time
(8 chunks x 128).  e_stack[(c,d), g*128+q] = E[d, g*1024+c*128+q]  (bf16)
m_t[q, g*48+c*6+k] = mask[k, g*1024+c*128+q]                       (bf16)

Pass 1 streams transposed E tiles (DMA xbar transpose) to accumulate
masked sums / counts on the TensorEngine and per-pixel |E|^2.
A small stats phase forms means / -2*means blockdiag / |mean|^2.
Pass 2 streams e_stack to build d2 = |E|^2 - 2 E.mu + |mu|^2 per
(pixel, instance) wholly in PSUM via 2 matmuls, then
G = relu(sqrt(relu(d2)*m) - 0.5) via one fused DVE op + 2 ACT ops, and
pull sums via G^T G matmuls (diagonal extracted on host).
"""

import numpy as np
import ml_dtypes

# ---- problem constants (hardcoded per contract) ----
B, D, H, W = 4, 16, 512, 512
K = 12
KH = 6                  # instances per core
NCORES = 8
HW = H * W              # 262144 pixels
P = 128                 # SBUF partitions
NCH = 8                 # pixel chunks per group
QP = 128                # pixels per chunk
GPX = NCH * QP          # 1024 pixels per group
NG = HW // GPX          # 256 groups
GM = 8                  # groups per macro tile
NMAC = NG // GM         # 32 macro iterations
CD = NCH * D            # 128   (c,d) stacked rows
CK = NCH * KH           # 48    (c,k) stacked cols
DELTA_PULL = 0.5
DELTA_PUSH = 1.5

_CACHE = {}


def _build_program():
    import concourse.bass as bass
    import concourse.tile as tile
    from concourse import bacc, mybir
    from contextlib import ExitStack

    import concourse.bass as _bass

    def _make_bcast_ap(src_ap):
        # [D, KH] -> [D, NCH(bcast), KH]
        return _bass.AP(
            tensor=src_ap.tensor, offset=src_ap.offset,
            ap=[src_ap.ap[0], [0, NCH], src_ap.ap[1]],
        )

    f32 = mybir.dt.float32
    bf16 = mybir.dt.bfloat16
    AX = mybir.AxisListType
    OP = mybir.AluOpType
    AF = mybir.ActivationFunctionType

    nc = bacc.Bacc()

    e_stack = nc.declare_dram_parameter("e_stack", [P, NG * QP], bf16, isOutput=False)
    m_t_d = nc.declare_dram_parameter("m_t", [P, NG * CK], bf16, isOutput=False)
    ident6_d = nc.declare_dram_parameter("ident6", [KH, KH], f32, isOutput=False)
    tiled16_d = nc.declare_dram_parameter("tiled16", [D, CD], f32, isOutput=False)
    tid_cd_d_d = nc.declare_dram_parameter("tid_cd_d", [CD, D], f32, isOutput=False)
    tid6_d = nc.declare_dram_parameter("tid6", [CK, KH], f32, isOutput=False)
    blockmask_d = nc.declare_dram_parameter("blockmask", [CD, CK], bf16, isOutput=False)
    identp_d = nc.declare_dram_parameter("identp", [P, P], bf16, isOutput=False)
    out_pull = nc.declare_dram_parameter("out_pull", [CK, CK], f32, isOutput=True)
    out_counts = nc.declare_dram_parameter("out_counts", [CK, 1], f32, isOutput=True)
    out_means = nc.declare_dram_parameter("out_means", [KH, D], f32, isOutput=True)

    with ExitStack() as ctx:
        tc = ctx.enter_context(tile.TileContext(nc))
        persist = ctx.enter_context(tc.tile_pool(name="persist", bufs=1))
        rot = ctx.enter_context(tc.tile_pool(name="rot", bufs=3))

        chain = ctx.enter_context(tc.tile_pool(name="chain", bufs=2))
        psum_per = ctx.enter_context(tc.tile_pool(name="psum_per", bufs=1, space="PSUM"))
        psum_rot = ctx.enter_context(tc.tile_pool(name="psum_rot", bufs=2, space="PSUM"))
        psum_tiny = ctx.enter_context(tc.tile_pool(name="psum_tiny", bufs=1, space="PSUM"))

        # persistent tiles
        e_res = persist.tile([P, NG * QP], bf16)        # resident e_stack
        identp = persist.tile([P, P], bf16)
        m_t_res = persist.tile([P, NG * CK], bf16)      # all masks, pixel-major
        embsq = persist.tile([P, NG * NCH], f32)        # per-pixel |E|^2, col g*8+c
        ones_q = persist.tile([P, 1], bf16)
        ones_row = persist.tile([1, P], f32)
        bd = persist.tile([CD, CK], bf16)               # blockdiag(-2*means)
        msq_row = persist.tile([1, CK], f32)            # |mean_k|^2 tiled over c
        neg_delta = persist.tile([P, 1], f32)
        ident6 = persist.tile([KH, KH], f32)
        tiled16 = persist.tile([D, CD], f32)
        tid_cd_d = persist.tile([CD, D], f32)
        tid6 = persist.tile([CK, KH], f32)
        blockmask = persist.tile([CD, CK], bf16)
        warm = persist.tile([1, 1], f32)

        nc.vector.memset(ones_q[:], 1.0)
        nc.vector.memset(ones_row[:], 1.0)
        nc.vector.memset(neg_delta[:], -DELTA_PULL)
        nc.sync.dma_start(ident6[:], ident6_d[:])
        nc.sync.dma_start(tiled16[:], tiled16_d[:])
        nc.sync.dma_start(tid_cd_d[:], tid_cd_d_d[:])
        nc.sync.dma_start(tid6[:], tid6_d[:])
        nc.sync.dma_start(blockmask[:], blockmask_d[:])
        nc.sync.dma_start(identp[:], identp_d[:])
        # engine warm-ups: make ACT/PE observe the const/memset ticks once so
        # later instructions need at most 2 sync waits (ISA limit).
        nc.scalar.activation(warm[:], ones_row[0:1, 0:1], AF.Square)

        psum_sums = psum_per.tile([CD, CK], f32)
        psum_counts = psum_per.tile([CK, 1], f32)
        psum_pull = psum_per.tile([CK, CK], f32)

        # ---------------- bulk loads (all DMA up front, dependency-free) ----
        NEQ = 8
        for i in range(NEQ):
            s = slice(i * NG * QP // NEQ, (i + 1) * NG * QP // NEQ)
            nc.sync.dma_start(e_res[:, s], e_stack[:, s])
        NMQ = 4
        for i in range(NMQ):
            s = slice(i * NG * CK // NMQ, (i + 1) * NG * CK // NMQ)
            nc.sync.dma_start(m_t_res[:, s], m_t_d[:, s])

        # ---------------- pass 1: masked sums / counts / |E|^2 ----------------
        for m in range(NMAC):
            e_t = rot.tile([P, GM, CD], bf16, tag="e_t")    # [q, g, (c,d)]
            for g in range(GM):
                gg = m * GM + g
                tps = psum_rot.tile([P, P], bf16, tag="pP")
                nc.tensor.transpose(
                    tps[:], e_res[:, gg * QP:(gg + 1) * QP], identp[:]
                )
                if g % 2 == 0:
                    nc.vector.tensor_copy(e_t[:, g, :], tps[:])
                else:
                    nc.scalar.activation(e_t[:, g, :], tps[:], AF.Copy)

            e_t_flat = e_t[:].rearrange("p g r -> p (g r)")
            sq_t = rot.tile([P, GM * CD], bf16, tag="sq_t")
            nc.scalar.activation(sq_t[:], e_t_flat, AF.Square)
            nc.vector.tensor_reduce(
                out=embsq[:, m * GM * NCH:(m + 1) * GM * NCH],
                in_=sq_t[:].rearrange("p (a d) -> p a d", d=D),
                axis=AX.X,
                op=OP.add,
            )
            for g in range(GM):
                gg = m * GM + g
                mgs = slice(gg * CK, (gg + 1) * CK)
                nc.tensor.matmul(
                    psum_sums[:], e_t[:, g, :], m_t_res[:, mgs],
                    start=(gg == 0), stop=(gg == NG - 1),
                )
                nc.tensor.matmul(
                    psum_counts[:], m_t_res[:, mgs], ones_q[:],
                    start=(gg == 0), stop=(gg == NG - 1),
                )

        # ---------------- stats: means, blockdiag, |mean|^2 ----------------
        # (no cross-partition DMAs: fold diag blocks with masks + tiny matmuls)
        sums_sb = persist.tile([CD, CK], f32)
        nc.vector.tensor_copy(sums_sb[:], psum_sums[:])
        counts_sb = persist.tile([CK, 1], f32)
        nc.vector.tensor_copy(counts_sb[:], psum_counts[:])

        # keep only diagonal (c) blocks, fold over c in the free dim
        s_diag = persist.tile([CD, CK], f32)
        nc.vector.tensor_mul(s_diag[:], sums_sb[:], blockmask[:])
        s_fold = persist.tile([CD, KH], f32)
        nc.vector.tensor_reduce(
            out=s_fold[:],
            in_=s_diag[:].rearrange("p (c k) -> p k c", c=NCH),
            axis=AX.X, op=OP.add,
        )
        # sums_kd[k, d] = sum_c s_fold[(c,d), k] via tiled-identity contraction
        psum_kd = psum_tiny.tile([KH, D], f32, tag="ptx")
        nc.tensor.matmul(psum_kd[:], s_fold[:], tid_cd_d[:], start=True, stop=True)

        # counts_k[k] = sum_c counts[(c,k)]
        psum_ck = psum_tiny.tile([KH, 1], f32, tag="pty")
        nc.tensor.matmul(psum_ck[:], tid6[:], counts_sb[:], start=True, stop=True)
        counts_k = persist.tile([KH, 1], f32)
        nc.vector.tensor_copy(counts_k[:], psum_ck[:])
        safe_k = persist.tile([KH, 1], f32)
        nc.vector.tensor_scalar_max(safe_k[:], counts_k[:], 1.0)
        recip_k = persist.tile([KH, 1], f32)
        nc.vector.reciprocal(recip_k[:], safe_k[:])

        means_kd = persist.tile([KH, D], f32)
        nc.vector.tensor_scalar(
            out=means_kd[:], in0=psum_kd[:], scalar1=recip_k[:], scalar2=None,
            op0=OP.mult,
        )
        nc.gpsimd.dma_start(out_means[:], means_kd[:])

        # means_dk = means_kd.T (PE transpose, base partition 0)
        psum_dk = psum_tiny.tile([D, KH], f32, tag="ptx")
        nc.tensor.transpose(psum_dk[:], means_kd[:], ident6[:])
        mdk_sb = persist.tile([D, KH], f32)
        nc.vector.tensor_scalar(
            out=mdk_sb[:], in0=psum_dk[:], scalar1=-2.0, scalar2=None, op0=OP.mult
        )
        # bd = blockdiag(-2*means): dense replicate via matmul, then mask
        psum_dense = psum_tiny.tile([CD, CK], f32, tag="pty")
        src_ap = mdk_sb[:]
        mdk_b = _make_bcast_ap(src_ap)
        nc.tensor.matmul(psum_dense[:], tiled16[:], mdk_b, start=True, stop=True)
        nc.vector.tensor_mul(bd[:], psum_dense[:], blockmask[:])

        # msq_row[0, (c,k)] = |mean_k|^2
        msq_t = persist.tile([KH, D], f32)
        nc.vector.tensor_mul(msq_t[:], means_kd[:], means_kd[:])
        msq_k = persist.tile([KH, 1], f32)
        nc.vector.tensor_reduce(out=msq_k[:], in_=msq_t[:], axis=AX.X, op=OP.add)
        psum_mr = psum_tiny.tile([1, KH], f32, tag="ptx")
        nc.tensor.transpose(psum_mr[:], msq_k[:], ident6[:])
        mr_src = psum_mr[:]
        mr_b = _bass.AP(
            tensor=mr_src.tensor, offset=mr_src.offset,
            ap=[mr_src.ap[0], [0, NCH], mr_src.ap[1]],
        )
        nc.vector.tensor_copy(msq_row[:].rearrange("p (c k) -> p c k", c=NCH), mr_b)

        # ---------------- pass 2: d2 -> pull sums ----------------
        for m in range(NMAC):
            msl = slice(m * GM * CK, (m + 1) * GM * CK)

            pP = psum_rot.tile([P, GM * CK], f32, tag="pP")
            for g in range(GM):
                gg = m * GM + g
                sl = slice(g * CK, (g + 1) * CK)
                nc.tensor.matmul(
                    pP[:, sl], ones_row[:], msq_row[:], start=True, stop=False
                )
                nc.tensor.matmul(
                    pP[:, sl], e_res[:, gg * QP:(gg + 1) * QP], bd[:],
                    start=False, stop=True,
                )

            # t = d2 = P + embsq (broadcast over k)
            eb = embsq[:, m * GM * NCH:(m + 1) * GM * NCH]
            eb_b = _bass.AP(
                tensor=eb.tensor, offset=eb.offset,
                ap=[eb.ap[0], eb.ap[1], [0, KH]],
            )  # [p, (g c), k]
            t_t = chain.tile([P, GM * CK], f32, tag="t_t")
            nc.vector.scalar_tensor_tensor(
                out=t_t[:].rearrange("p (a k) -> p a k", k=KH),
                in0=pP[:].rearrange("p (a k) -> p a k", k=KH),
                scalar=0.0, in1=eb_b, op0=OP.bypass, op1=OP.add,
            )
            # u = relu(d2) * m
            u_t = chain.tile([P, GM * CK], f32, tag="u_t")
            nc.vector.scalar_tensor_tensor(
                out=u_t[:], in0=t_t[:], scalar=0.0, in1=m_t_res[:, msl],
                op0=OP.max, op1=OP.mult,
            )
            w_t = chain.tile([P, GM * CK], f32, tag="w_t")
            nc.scalar.sqrt(w_t[:], u_t[:])
            g_t = chain.tile([P, GM * CK], f32, tag="g_t")
            nc.scalar.activation(g_t[:], w_t[:], AF.Relu, bias=neg_delta[:])
            for g in range(GM):
                gg = m * GM + g
                sl = slice(g * CK, (g + 1) * CK)
                nc.tensor.matmul(
                    psum_pull[:], g_t[:, sl], g_t[:, sl],
                    start=(gg == 0), stop=(gg == NG - 1),
                )

        pull_sb = persist.tile([CK, CK], f32)
        nc.vector.tensor_copy(pull_sb[:], psum_pull[:])
        nc.gpsimd.dma_start(out_pull[:], pull_sb[:])
        nc.gpsimd.dma_start(out_counts[:], counts_sb[:])

    nc.finalize()
    return nc


def _get_program():
    if "nc" not in _CACHE:
        _CACHE["nc"] = _build_program()
    return _CACHE["nc"]


def _host_consts():
    bf = ml_dtypes.bfloat16
    id16 = np.eye(D, dtype=np.float32)
    id6 = np.eye(KH, dtype=np.float32)
    tiled16 = np.tile(id16, (1, NCH)).astype(np.float32)          # [16, 128]
    tid_cd_d = np.tile(id16, (NCH, 1)).astype(np.float32)         # [128, 16]
    tid6 = np.tile(id6, (NCH, 1)).astype(np.float32)              # [48, 6]
    blockmask = np.zeros((CD, CK), dtype=np.float32)
    for c in range(NCH):
        blockmask[c * D:(c + 1) * D, c * KH:(c + 1) * KH] = 1.0
    return {
        "ident6": id6,
        "tiled16": tiled16,
        "tid_cd_d": tid_cd_d,
        "tid6": tid6,
        "blockmask": blockmask.astype(bf),
        "identp": np.eye(P, dtype=np.float32).astype(bf),
    }


def _prep_core_inputs(emb_b, masks_bh):
    """emb_b: [16, H, W] f32; masks_bh: [6, H, W] (int or float)."""
    bf = ml_dtypes.bfloat16
    e = np.ascontiguousarray(emb_b.reshape(D, NG, NCH, QP).transpose(2, 0, 1, 3))
    e_stack = e.reshape(P, NG * QP).astype(bf)
    mk = masks_bh.reshape(KH, NG, NCH, QP).transpose(3, 1, 2, 0)  # [q, g, c, k]
    m_t = np.ascontiguousarray(mk).reshape(P, NG * CK).astype(bf)
    return {"e_stack": e_stack, "m_t": m_t, **_host_consts()}


def _host_combine(core_results):
    """core_results: list of 8 dicts with out_pull [48,48], out_counts [48,1],
    out_means [6,16]. Returns np.float32 scalar total loss."""
    total = 0.0
    for b in range(B):
        means = []
        counts = []
        pull_sums = []
        for h in range(2):
            r = core_results[b * 2 + h]
            pull_k = np.diag(r["out_pull"].astype(np.float64)).reshape(NCH, KH).sum(0)
            cnt_k = r["out_counts"].astype(np.float64).reshape(NCH, KH).sum(0)
            means.append(r["out_means"].astype(np.float64))
            counts.append(cnt_k)
            pull_sums.append(pull_k)
        means = np.concatenate(means, 0)          # [12, 16]
        counts = np.concatenate(counts, 0)        # [12]
        pull_sums = np.concatenate(pull_sums, 0)  # [12]

        valid = counts > 0
        validf = valid.astype(np.float64)
        safe_counts = np.maximum(counts, 1.0)
        nv = validf.sum()
        safe_nv = max(nv, 1.0)

        pull_k = pull_sums / safe_counts
        pull_img = (pull_k * validf).sum() / safe_nv if nv > 0 else 0.0

        mean_sq = (means * means).sum(-1)                      # [12]
        cross = means @ means.T                                # [12,12]
        pd2 = np.maximum(mean_sq[:, None] + mean_sq[None, :] - 2.0 * cross, 0.0)
        iu = np.triu_indices(K, k=1)
        pair_mask = (valid[:, None] & valid[None, :])[iu]
        pdist = np.sqrt(pd2[iu])
        push_terms = np.where(
            pair_mask, np.maximum(2.0 * DELTA_PUSH - pdist, 0.0) ** 2, 0.0
        )
        n_pairs = nv * (nv - 1.0) / 2.0
        push_img = push_terms.sum() / max(n_pairs, 1.0) if nv > 1 else 0.0

        reg_img = (np.sqrt(mean_sq) * validf).sum() / safe_nv if nv > 0 else 0.0

        total += pull_img + push_img + reg_img
    return np.float32(total / B)


def _get_runner():
    """Build the program once and return a cached jitted SPMD executor.

    Mirrors concourse.bass2jax.run_bass_via_pjrt's multi-core branch, but
    caches the jitted callable so repeated kernel() calls don't re-lower.
    """
    if "runner" in _CACHE:
        return _CACHE["runner"]

    import jax
    import jax.numpy as jnp
    from jax.sharding import Mesh, PartitionSpec
    from jax.experimental.shard_map import shard_map
    from concourse import bass2jax, mybir
    from concourse.bass2jax import _bass_exec_p, partition_id_tensor

    nc = _get_program()
    bass2jax.install_neuronx_cc_hook()

    in_names, out_names, out_avals, zero_outs = [], [], [], []
    partition_name = nc.partition_id_tensor.name if nc.partition_id_tensor else None
    for alloc in nc.m.functions[0].allocations:
        if not isinstance(alloc, mybir.MemoryLocationSet):
            continue
        name = alloc.memorylocations[0].name
        if alloc.kind == "ExternalInput":
            if name != partition_name:
                in_names.append(name)
        elif alloc.kind == "ExternalOutput":
            out_names.append(name)
            shape = tuple(alloc.tensor_shape)
            dtype = mybir.dt.np(alloc.dtype)
            out_avals.append(jax.core.ShapedArray(shape, dtype))
            zero_outs.append(np.zeros(shape, dtype))
    n_params = len(in_names)
    n_outs = len(out_avals)
    all_in_names = tuple(in_names + out_names + ([partition_name] if partition_name else []))

    def _body(*args):
        operands = list(args)
        if partition_name is not None:
            operands.append(partition_id_tensor())
        outs = _bass_exec_p.bind(
            *operands,
            out_avals=tuple(out_avals),
            in_names=all_in_names,
            out_names=tuple(out_names),
            lowering_input_output_aliases=(),
            sim_require_finite=True,
            sim_require_nnan=True,
            nc=nc,
        )
        return tuple(outs)

    devices = jax.devices()[:NCORES]
    mesh = Mesh(np.asarray(devices), ("core",))
    in_specs = (PartitionSpec("core"),) * (n_params + n_outs)
    out_specs = (PartitionSpec("core"),) * n_outs
    donate = tuple(range(n_params, n_params + n_outs))
    sharded = jax.jit(
        shard_map(_body, mesh=mesh, in_specs=in_specs, out_specs=out_specs,
                  check_rep=False),
        donate_argnums=donate, keep_unused=True,
    )

    runner = {
        "fn": sharded, "in_names": in_names, "out_names": out_names,
        "out_avals": out_avals, "zero_outs": zero_outs,
    }
    _CACHE["runner"] = runner
    return runner


def _concat_inputs(in_maps, runner):
    return [
        np.concatenate([in_maps[c][name] for c in range(NCORES)], axis=0)
        for name in runner["in_names"]
    ]


def _zero_globals(runner):
    return [np.zeros((NCORES * z.shape[0], *z.shape[1:]), z.dtype)
            for z in runner["zero_outs"]]


def _split_outputs(out_arrs, runner):
    res = []
    for c in range(NCORES):
        res.append({
            name: np.asarray(out_arrs[i]).reshape(
                NCORES, *runner["out_avals"][i].shape)[c]
            for i, name in enumerate(runner["out_names"])
        })
    return res


def _make_in_maps(embedding_maps, instance_masks):
    emb = np.asarray(embedding_maps, dtype=np.float32)
    msk = np.asarray(instance_masks)
    in_maps = []
    for core in range(NCORES):
        b, h = core // 2, core % 2
        in_maps.append(
            _prep_core_inputs(emb[b], msk[b, h * KH:(h + 1) * KH].astype(np.float32))
        )
    return in_maps


def kernel(embedding_maps, instance_masks):
    runner = _get_runner()
    in_maps = _make_in_maps(embedding_maps, instance_masks)
    out_arrs = runner["fn"](*_concat_inputs(in_maps, runner), *_zero_globals(runner))
    return _host_combine(_split_outputs(out_arrs, runner))


if __name__ == "__main__":
    rng = np.random.default_rng(0)
    emb = rng.standard_normal((B, D, H, W), dtype=np.float32)
    msk = (rng.random((B, K, H, W)) < 0.5).astype(np.int32)
    print(kernel(emb, msk))
